# revision 1
# baseline (speedup 1.0000x reference)
"""Trainium2 Bass kernel for a dense transformer block (B=2, T=2048, C=1024, H=16).

Sharding (8 NeuronCores, one chip; identical instruction stream per core,
per-core differences enter only through input data):
  - LayerNorms / projections / MLP: token-sharded. 4096 tokens -> 512 per core.
    Core c owns 128-token blocks {c, 15-c} of each batch (causal load balance).
  - Attention: head-sharded. Core c computes heads {c, c+8} for both batches
    over the full causal sequence.
  - Collective glue: AllGather of h1^T (post-LN1 activations, 1 MB/rank) and
    AllToAll of o^T (attention output, 1 MB/rank). No AllReduce.

Precision: matmul operands are bf16 (PE runs 4x faster than fp32: fp32 is
2 half-rate passes = 4 cycles/row), accumulation is always fp32 in PSUM;
LayerNorm statistics, softmax normalization and both residual streams stay
fp32.

Layout notes:
  - All matmul contractions keep the contraction dim on SBUF partitions.
  - Attention computes scores transposed: ST[s,t] = K @ Q^T per head so that
    P@V contracts over s (partitions) directly; softmax uses un-max-subtracted
    exp (scores measured in [-3.2, 3.3]) with a ones-column appended to V to
    produce row sums in the same matmul accumulation.
"""

import sys

if "/opt/trn_rl_repo" not in sys.path:
    sys.path.insert(0, "/opt/trn_rl_repo")

import ml_dtypes
import numpy as np

import concourse.bass as bass
import concourse.mybir as mybir
import concourse.tile as tile
from concourse import bacc
from concourse.bass_utils import run_bass_kernel_spmd

FP = mybir.dt.float32
BF = mybir.dt.bfloat16
NPBF = ml_dtypes.bfloat16
AF = mybir.ActivationFunctionType
ALU = mybir.AluOpType

B, T, C, H, HD = 2, 2048, 1024, 16, 64
NCORE = 8
BLK = 128
NB = T // BLK  # 16 blocks of 128 tokens per batch
OWN = B * T // NCORE  # 512 tokens per core
EPS = 1e-5

# Optional knobs for the local test harness (not used by grader)
TRACE = False
LAST_RESULT = None
SIM_MODE = False  # replace collectives with local DMA copies (TimelineSim)


def _own_blocks(c):
    """Blocks (b, j) owned by core c, in shard-row order."""
    return [(b, j) for b in range(B) for j in (c, NB - 1 - c)]


def _rank_of(j):
    return j if j < NCORE else NB - 1 - j


def _col_in_rank(b, j):
    return b * 2 * BLK + (0 if j < NCORE else BLK)


def _gcol(b, j):
    """Column of natural block (b, j) in rank-major gathered token order."""
    return _rank_of(j) * OWN + _col_in_rank(b, j)


def _vidx(b, j):
    """Index of block (b, j) in the v tile array [32] (rank-major)."""
    return _rank_of(j) * 4 + b * 2 + (0 if j < NCORE else 1)


def _bcast(handle, n_free):
    """AP broadcasting a 1-D DRAM tensor across 128 partitions (DMA only)."""
    ap = handle[:]
    return bass.AP(tensor=ap.tensor, offset=ap.offset, ap=[[0, 128], *ap.ap])


def _layernorm(nc, pool_stats, eps_sb, out_ap, in_ap, g_sb, be_sb):
    """LN over free axis (1024) of a [128, 1024] tile; out may alias in_."""
    x3 = in_ap.rearrange("p (n s) -> p n s", s=512)
    stats = pool_stats.tile([128, 2, 6], FP, tag="bnstats")
    for sg in range(2):
        nc.vector.bn_stats(out=stats[:, sg, :], in_=x3[:, sg, :])
    mv = pool_stats.tile([128, 2], FP, tag="bnaggr")
    nc.vector.bn_aggr(out=mv, in_=stats)
    std = pool_stats.tile([128, 1], FP, tag="std")
    nc.scalar.activation(out=std, in_=mv[:, 1:2], func=AF.Sqrt, bias=eps_sb)
    rstd = pool_stats.tile([128, 1], FP, tag="rstd")
    nc.vector.reciprocal(out=rstd, in_=std)
    nc.vector.tensor_scalar(
        out=out_ap,
        in0=in_ap,
        scalar1=mv[:, 0:1],
        scalar2=rstd,
        op0=ALU.subtract,
        op1=ALU.mult,
    )
    if g_sb is not None:
        nc.vector.tensor_mul(out=out_ap, in0=out_ap, in1=g_sb)
    if be_sb is not None:
        nc.vector.tensor_add(out=out_ap, in0=out_ap, in1=be_sb)


def _build(reps=1, ln1_affine=True, ln2_affine=True, add_b2=True, add_bproj=True):
    nc = bacc.Bacc(None, num_devices=NCORE)

    # ---- kernel I/O (per-core data differs, shapes identical) ----
    x_own = nc.dram_tensor("x_own", [OWN, C], FP, kind="ExternalInput")
    wq = nc.dram_tensor("wq", [C, 2 * HD], BF, kind="ExternalInput")
    wk = nc.dram_tensor("wk", [C, 2 * HD], BF, kind="ExternalInput")
    wv = nc.dram_tensor("wv", [C, 2 * HD], BF, kind="ExternalInput")
    wproj = nc.dram_tensor("wproj", [C, C], BF, kind="ExternalInput")
    w1b = nc.dram_tensor("w1b", [32, C, 128], BF, kind="ExternalInput")
    w2 = nc.dram_tensor("w2", [4 * C, C], BF, kind="ExternalInput")
    b1v = nc.dram_tensor("b1v", [32, 128], FP, kind="ExternalInput")
    bproj = nc.dram_tensor("bproj", [C], FP, kind="ExternalInput")
    b2 = nc.dram_tensor("b2", [C], FP, kind="ExternalInput")
    g1 = nc.dram_tensor("g1", [C], FP, kind="ExternalInput")
    be1 = nc.dram_tensor("be1", [C], FP, kind="ExternalInput")
    g2 = nc.dram_tensor("g2", [C], FP, kind="ExternalInput")
    be2 = nc.dram_tensor("be2", [C], FP, kind="ExternalInput")
    utri = nc.dram_tensor("utri", [BLK, BLK], BF, kind="ExternalInput")
    ident = nc.dram_tensor("ident", [BLK, BLK], FP, kind="ExternalInput")
    out = nc.dram_tensor("out", [OWN, C], FP, kind="ExternalOutput")

    rg = [list(range(NCORE))]

    with tile.TileContext(nc) as tc:
        with (
            tc.tile_pool(name="dram", bufs=1, space="DRAM") as dram,
            tc.tile_pool(name="consts", bufs=1) as consts,
            tc.tile_pool(name="stats", bufs=12) as stats,
            tc.tile_pool(name="resid", bufs=4) as resid,
            tc.tile_pool(name="tp_ps", bufs=2, space="PSUM") as tp_ps,
        ):

            # ---- constants in SBUF ----
            eps_sb = consts.tile([128, 1], FP)
            nc.vector.memset(eps_sb, EPS)
            g1b = consts.tile([128, C], FP)
            nc.gpsimd.dma_start(out=g1b, in_=_bcast(g1, C))
            be1b = consts.tile([128, C], FP)
            nc.gpsimd.dma_start(out=be1b, in_=_bcast(be1, C))
            g2b = consts.tile([128, C], FP)
            nc.gpsimd.dma_start(out=g2b, in_=_bcast(g2, C))
            be2b = consts.tile([128, C], FP)
            nc.gpsimd.dma_start(out=be2b, in_=_bcast(be2, C))
            bprojb = consts.tile([128, C], FP)
            nc.gpsimd.dma_start(out=bprojb, in_=_bcast(bproj, C))
            b2b = consts.tile([128, C], FP)
            nc.gpsimd.dma_start(out=b2b, in_=_bcast(b2, C))
            utri_sb = consts.tile([BLK, BLK], BF)
            nc.sync.dma_start(out=utri_sb, in_=utri[:])
            utri2_sb = consts.tile([BLK, 2 * BLK], BF)
            nc.vector.tensor_copy(out=utri2_sb[:, 0:BLK], in_=utri_sb)
            nc.vector.tensor_copy(out=utri2_sb[:, BLK : 2 * BLK], in_=utri_sb)
            ident_sb = consts.tile([BLK, BLK], FP)
            nc.sync.dma_start(out=ident_sb, in_=ident[:])
            b1_sb = consts.tile([128, 32], FP)
            nc.gpsimd.dma_start(out=b1_sb, in_=b1v[:].rearrange("a p -> p a"))

            def _body(rep):
                h1T_shard = dram.tile([C, OWN], BF, name=f"h1T_shard{rep}", tag=f"sh{rep}")
                h1T_gatha = dram.tile(
                    [NCORE * (C // 2), OWN], BF, name=f"h1T_gatha{rep}", tag=f"ga{rep}",
                    addr_space="Local" if SIM_MODE else "Shared",
                )
                h1T_gathb = dram.tile(
                    [NCORE * (C // 2), OWN], BF, name=f"h1T_gathb{rep}", tag=f"gb{rep}",
                    addr_space="Local" if SIM_MODE else "Shared",
                )
                # AllToAll: a2a_in rows r*128.. = my heads' o^T for rank r's
                # tokens; a2a_out rows r*128.. = rank r's heads for MY tokens.
                a2a_in = dram.tile([NCORE * BLK, OWN], BF, name=f"a2a_in{rep}", tag=f"ai{rep}")
                a2a_out = dram.tile([NCORE * BLK, OWN], BF, name=f"a2a_out{rep}", tag=f"ao{rep}")
                # ================= Phase 1: LN1 on own tokens, h1^T shard =======
                xo_sb = []  # own x tiles; overwritten with x2 (post-attn residual)
                for i in range(4):
                    xo = resid.tile([128, C], FP, tag="xo", name=f"xo{i}")
                    xo_sb.append(xo)
                with (
                    tc.tile_pool(name="hwork", bufs=4) as hwork,
                    tc.tile_pool(name="h1Tp", bufs=8) as h1Tp,
                ):
                    h1T_sb = [
                        h1Tp.tile([128, OWN], BF, tag="h1T", name=f"h1T{ct}")
                        for ct in range(8)
                    ]
                    for i in range(4):
                        h1 = hwork.tile([128, C], FP, tag="h1", name=f"h1_{i}")
                        nc.sync.dma_start(
                            out=xo_sb[i], in_=x_own[i * 128 : (i + 1) * 128, :]
                        )
                        _layernorm(nc, stats, eps_sb, h1[:], xo_sb[i][:], g1b, be1b)
                        for ct in range(8):
                            tp = tp_ps.tile([128, 128], FP, tag="tp", name="tp1")
                            nc.tensor.transpose(
                                tp, h1[:, ct * 128 : (ct + 1) * 128], ident_sb
                            )
                            dst = h1T_sb[ct][:, i * 128 : (i + 1) * 128]
                            if ct % 2 == 0:
                                nc.vector.tensor_copy(out=dst, in_=tp)
                            else:
                                nc.scalar.copy(out=dst, in_=tp)
                    for ct in range(8):
                        nc.sync.dma_start(
                            out=h1T_shard[ct * 128 : (ct + 1) * 128, :], in_=h1T_sb[ct]
                        )

                # ================= Phase 2: AllGather h1^T (split) ==============
                HC = C // 2
                if SIM_MODE:
                    for r in range(NCORE):
                        nc.sync.dma_start(
                            out=h1T_gatha[r * HC : (r + 1) * HC, :],
                            in_=h1T_shard[0:HC, :],
                        )
                        nc.sync.dma_start(
                            out=h1T_gathb[r * HC : (r + 1) * HC, :],
                            in_=h1T_shard[HC:C, :],
                        )
                else:
                    nc.gpsimd.collective_compute(
                        "AllGather",
                        ALU.bypass,
                        replica_groups=rg,
                        ins=[h1T_shard[0:HC, :].opt()],
                        outs=[h1T_gatha[:].opt()],
                    )
                    nc.gpsimd.collective_compute(
                        "AllGather",
                        ALU.bypass,
                        replica_groups=rg,
                        ins=[h1T_shard[HC:C, :].opt()],
                        outs=[h1T_gathb[:].opt()],
                    )

                # ================= Phase 3: QKV for own heads, all tokens =======
                with (
                    tc.tile_pool(name="wqkv", bufs=1) as wqkv,
                    tc.tile_pool(name="h1Tin", bufs=16) as h1Tin,
                    tc.tile_pool(name="attn_res", bufs=1) as attn_res,
                ):
                    wq_sb = wqkv.tile([128, 8, 2 * HD], BF, tag="wq")
                    nc.gpsimd.dma_start(
                        out=wq_sb, in_=wq[:].rearrange("(a p) m -> p a m", p=128)
                    )
                    wk_sb = wqkv.tile([128, 8, 2 * HD], BF, tag="wk")
                    nc.gpsimd.dma_start(
                        out=wk_sb, in_=wk[:].rearrange("(a p) m -> p a m", p=128)
                    )
                    wv_sb = wqkv.tile([128, 8, 2 * HD], BF, tag="wv")
                    nc.gpsimd.dma_start(
                        out=wv_sb, in_=wv[:].rearrange("(a p) m -> p a m", p=128)
                    )

                    qT_sb = attn_res.tile([128, B * T], BF, tag="qT")
                    kT_sb = attn_res.tile([128, B * T], BF, tag="kT")
                    vv_sb = attn_res.tile([128, 32, 130], BF, tag="vv")

                    qkv_ps_ctx = tc.tile_pool(name="qkv_ps", bufs=2, space="PSUM")
                    qkv_ps = qkv_ps_ctx.__enter__()
                    for r in range(NCORE):
                        hts = []
                        for ct in range(8):
                            ht = h1Tin.tile([128, OWN], BF, tag="ht", name=f"ht{r}_{ct}")
                            gsrc = h1T_gatha if ct < 4 else h1T_gathb
                            goff = r * (C // 2) + (ct % 4) * 128
                            nc.sync.dma_start(
                                out=ht, in_=gsrc[goff : goff + 128, :]
                            )
                            hts.append(ht)
                        q_ps = qkv_ps.tile([128, OWN], FP, tag="q_ps")
                        for ct in range(8):
                            nc.tensor.matmul(
                                q_ps, wq_sb[:, ct, :], hts[ct],
                                start=(ct == 0), stop=(ct == 7),
                            )
                        nc.vector.tensor_copy(
                            out=qT_sb[:, r * OWN : (r + 1) * OWN], in_=q_ps
                        )
                        k_ps = qkv_ps.tile([128, OWN], FP, tag="k_ps")
                        for ct in range(8):
                            nc.tensor.matmul(
                                k_ps, wk_sb[:, ct, :], hts[ct],
                                start=(ct == 0), stop=(ct == 7),
                            )
                        nc.vector.tensor_copy(
                            out=kT_sb[:, r * OWN : (r + 1) * OWN], in_=k_ps
                        )
                        for sub in range(4):
                            v_ps = qkv_ps.tile([128, 2 * HD], FP, tag="v_ps")
                            for ct in range(8):
                                nc.tensor.matmul(
                                    v_ps,
                                    hts[ct][:, sub * 128 : (sub + 1) * 128],
                                    wv_sb[:, ct, :],
                                    start=(ct == 0), stop=(ct == 7),
                                )
                            vi = r * 4 + sub
                            nc.vector.tensor_copy(
                                out=vv_sb[:, vi, 0:HD], in_=v_ps[:, 0:HD]
                            )
                            nc.vector.tensor_copy(
                                out=vv_sb[:, vi, HD + 1 : 2 * HD + 1],
                                in_=v_ps[:, HD : 2 * HD],
                            )
                            nc.vector.memset(vv_sb[:, vi, HD : HD + 1], 1.0)
                            nc.vector.memset(vv_sb[:, vi, 2 * HD + 1 : 2 * HD + 2], 1.0)
                    qkv_ps_ctx.__exit__(None, None, None)

                    # ============= Phase 4: causal attention, own heads =========
                    with (
                        tc.tile_pool(name="st_ps", bufs=3, space="PSUM") as st_ps,
                        tc.tile_pool(name="o_ps", bufs=3, space="PSUM") as o_ps_pool,
                        tc.tile_pool(name="pt", bufs=26) as pt_pool,
                        tc.tile_pool(name="oblk", bufs=4) as oblk_pool,
                        tc.tile_pool(name="otsb", bufs=4) as ot_pool,
                    ):
                        for jq in range(NB):
                            for b in range(B):
                                qsl = slice(_gcol(b, jq), _gcol(b, jq) + BLK)
                                oblk = oblk_pool.tile([128, 128], FP, tag="oblk")
                                pts = []
                                for j in range(jq + 1):
                                    ksl = slice(_gcol(b, j), _gcol(b, j) + BLK)
                                    pt2 = pt_pool.tile([128, 2 * BLK], BF, tag="pt")
                                    for hx in range(2):
                                        hs = slice(hx * HD, (hx + 1) * HD)
                                        st = st_ps.tile([128, BLK], FP, tag="st")
                                        nc.tensor.matmul(
                                            st, kT_sb[hs, ksl], qT_sb[hs, qsl],
                                            start=True, stop=True,
                                        )
                                        nc.scalar.activation(
                                            out=pt2[:, hx * BLK : (hx + 1) * BLK],
                                            in_=st, func=AF.Exp, scale=0.125,
                                        )
                                    if j == jq:
                                        nc.vector.tensor_mul(
                                            out=pt2, in0=pt2, in1=utri2_sb
                                        )
                                    pts.append((pt2, _vidx(b, j)))
                                for hx in range(2):
                                    o_ps = o_ps_pool.tile([128, HD + 1], FP, tag="o_ps")
                                    vsl = slice(hx * (HD + 1), (hx + 1) * (HD + 1))
                                    for i, (pt2, vi) in enumerate(pts):
                                        nc.tensor.matmul(
                                            o_ps,
                                            pt2[:, hx * BLK : (hx + 1) * BLK],
                                            vv_sb[:, vi, vsl],
                                            start=(i == 0), stop=(i == len(pts) - 1),
                                        )
                                    recip = stats.tile([128, 1], FP, tag="recip")
                                    nc.vector.reciprocal(
                                        out=recip, in_=o_ps[:, HD : HD + 1]
                                    )
                                    nc.vector.tensor_scalar_mul(
                                        out=oblk[:, hx * HD : (hx + 1) * HD],
                                        in0=o_ps[:, 0:HD],
                                        scalar1=recip,
                                    )
                                tp = tp_ps.tile([128, 128], FP, tag="tp", name="tp4")
                                nc.tensor.transpose(tp, oblk, ident_sb)
                                ot = ot_pool.tile([128, 128], BF, tag="ot")
                                nc.vector.tensor_copy(out=ot, in_=tp)
                                rt = _rank_of(jq)
                                co = _col_in_rank(b, jq)
                                nc.sync.dma_start(
                                    out=a2a_in[rt * BLK : (rt + 1) * BLK, co : co + BLK],
                                    in_=ot,
                                )

                # ================= Phase 4.5: AllToAll o^T ======================
                if SIM_MODE:
                    for r in range(NCORE):
                        nc.sync.dma_start(
                            out=a2a_out[r * BLK : (r + 1) * BLK, :],
                            in_=a2a_in[r * BLK : (r + 1) * BLK, :],
                        )
                else:
                    nc.gpsimd.collective_compute(
                        "AllToAll",
                        ALU.bypass,
                        replica_groups=rg,
                        ins=[a2a_in[:].opt()],
                        outs=[a2a_out[:].opt()],
                    )

                # ================= Phase 5: proj + LN2 + MLP on own tokens ======
                with (
                    tc.tile_pool(name="mm_ps", bufs=5, space="PSUM") as mm_ps,
                    tc.tile_pool(name="uT", bufs=32) as uT_pool,
                    tc.tile_pool(name="x3p", bufs=4) as x3_pool,
                ):
                    with (
                        tc.tile_pool(name="h2Tp", bufs=8) as h2T_pool,
                    ):
                        # --- attention projection + residual (into xo_sb) ---
                        with (
                            tc.tile_pool(name="wp", bufs=8) as wp_pool,
                            tc.tile_pool(name="oTg", bufs=8) as oTg_pool,
                            tc.tile_pool(name="hwork2", bufs=4) as hwork2,
                        ):
                            oTg_sb = []
                            wp_sb = []
                            for ct in range(8):
                                og = oTg_pool.tile([128, OWN], BF, tag="og", name=f"og{ct}")
                                nc.sync.dma_start(
                                    out=og, in_=a2a_out[ct * 128 : (ct + 1) * 128, :]
                                )
                                oTg_sb.append(og)
                                wp = wp_pool.tile([128, C], BF, tag="wp", name=f"wp{ct}")
                                nc.sync.dma_start(
                                    out=wp, in_=wproj[ct * 128 : (ct + 1) * 128, :]
                                )
                                wp_sb.append(wp)
                            for tq in range(4):
                                for co in range(2):
                                    ps = mm_ps.tile([128, 512], FP, tag="mm")
                                    for ct in range(8):
                                        nc.tensor.matmul(
                                            ps,
                                            oTg_sb[ct][:, tq * 128 : (tq + 1) * 128],
                                            wp_sb[ct][:, co * 512 : (co + 1) * 512],
                                            start=(ct == 0), stop=(ct == 7),
                                        )
                                    csl = slice(co * 512, (co + 1) * 512)
                                    nc.vector.tensor_add(
                                        out=xo_sb[tq][:, csl],
                                        in0=xo_sb[tq][:, csl],
                                        in1=ps,
                                    )
                                    if add_bproj:
                                        nc.vector.tensor_add(
                                            out=xo_sb[tq][:, csl],
                                            in0=xo_sb[tq][:, csl],
                                            in1=bprojb[:, csl],
                                        )

                            # --- LN2 + transpose to h2T ---
                            h2T_sb = [
                                h2T_pool.tile([128, OWN], BF, tag="h2T", name=f"h2T{ct}")
                                for ct in range(8)
                            ]
                            for tq in range(4):
                                h2 = hwork2.tile([128, C], FP, tag="h2", name=f"h2_{tq}")
                                _layernorm(
                                    nc, stats, eps_sb, h2[:], xo_sb[tq][:], g2b, be2b
                                )
                                for ct in range(8):
                                    tp = tp_ps.tile([128, 128], FP, tag="tp", name="tp5")
                                    nc.tensor.transpose(
                                        tp, h2[:, ct * 128 : (ct + 1) * 128], ident_sb
                                    )
                                    dst = h2T_sb[ct][:, tq * 128 : (tq + 1) * 128]
                                    if ct % 2 == 0:
                                        nc.vector.tensor_copy(out=dst, in_=tp)
                                    else:
                                        nc.scalar.copy(out=dst, in_=tp)

                        # --- MLP up: uT[ut] = relu(W1[:, ut].T @ h2T + b1) ---
                        with tc.tile_pool(name="w1s", bufs=4) as w1_pool:
                            uT_sb = []
                            for ut in range(32):
                                w1t = w1_pool.tile(
                                    [128, 8, 128], BF, tag="w1", name=f"w1_{ut}"
                                )
                                nc.sync.dma_start(
                                    out=w1t,
                                    in_=w1b[ut, :, :].rearrange("(a p) m -> p a m", p=128),
                                )
                                ups = mm_ps.tile([128, 512], FP, tag="mm")
                                for ct in range(8):
                                    nc.tensor.matmul(
                                        ups, w1t[:, ct, :], h2T_sb[ct],
                                        start=(ct == 0), stop=(ct == 7),
                                    )
                                u = uT_pool.tile([128, OWN], BF, tag="uT", name=f"uT{ut}")
                                nc.scalar.activation(
                                    out=u, in_=ups, func=AF.Relu,
                                    bias=b1_sb[:, ut : ut + 1],
                                )
                                uT_sb.append(u)

                    # --- MLP down + residual: out = x2 + uT.T @ W2 + b2 ---
                    with tc.tile_pool(name="w2s", bufs=10) as w2_pool:
                        x3_sb = []
                        for tq in range(4):
                            x3 = x3_pool.tile([128, C], FP, tag="x3", name=f"x3_{tq}")
                            if add_b2:
                                nc.vector.tensor_add(out=x3, in0=xo_sb[tq], in1=b2b)
                            x3_sb.append(x3)
                        for g in range(4):
                            w2_sb = []
                            for k in range(8):
                                ut = g * 8 + k
                                w2t = w2_pool.tile([128, C], BF, tag="w2", name=f"w2_{ut}")
                                nc.sync.dma_start(
                                    out=w2t, in_=w2[ut * 128 : (ut + 1) * 128, :]
                                )
                                w2_sb.append(w2t)
                            for tq in range(4):
                                for co in range(2):
                                    ps = mm_ps.tile([128, 512], FP, tag="mm")
                                    for k in range(8):
                                        nc.tensor.matmul(
                                            ps,
                                            uT_sb[g * 8 + k][:, tq * 128 : (tq + 1) * 128],
                                            w2_sb[k][:, co * 512 : (co + 1) * 512],
                                            start=(k == 0), stop=(k == 7),
                                        )
                                    csl = slice(co * 512, (co + 1) * 512)
                                    nc.vector.tensor_add(
                                        out=x3_sb[tq][:, csl],
                                        in0=(x3_sb[tq] if (add_b2 or g > 0) else xo_sb[tq])[:, csl],
                                        in1=ps,
                                    )
                        for tq in range(4):
                            nc.sync.dma_start(
                                out=out[tq * 128 : (tq + 1) * 128, :], in_=x3_sb[tq]
                            )

            for _rep in range(reps):
                _body(_rep)


    nc.compile()
    return nc


def _prep_inputs(inputs):
    """Host-side prep: returns per-core in_maps."""
    f32 = lambda a: np.ascontiguousarray(np.asarray(a, dtype=np.float32))
    bf = lambda a: np.ascontiguousarray(np.asarray(a, dtype=np.float32).astype(NPBF))
    x = f32(inputs["x"])
    Wq = np.asarray(inputs["Wq"], np.float32).transpose(1, 0, 2).reshape(C, C)
    Wk = np.asarray(inputs["Wk"], np.float32).transpose(1, 0, 2).reshape(C, C)
    Wv = np.asarray(inputs["Wv"], np.float32).transpose(1, 0, 2).reshape(C, C)
    Wproj = np.asarray(inputs["Wproj"], np.float32)
    W1 = np.asarray(inputs["W1"], np.float32)
    W2 = np.asarray(inputs["W2"], np.float32)

    # permute Wproj rows into gathered-o^T channel order (rank-major heads)
    perm = np.concatenate(
        [np.r_[r * HD : (r + 1) * HD, (r + 8) * HD : (r + 9) * HD] for r in range(8)]
    )
    Wproj_p = bf(Wproj[perm, :])
    W1b = bf(W1.reshape(C, 32, 128).transpose(1, 0, 2))
    b1v = np.ascontiguousarray(np.asarray(inputs["b1"], np.float32).reshape(32, 128))
    utri_m = np.ascontiguousarray(np.triu(np.ones((BLK, BLK), np.float32)).astype(NPBF))
    ident_m = np.ascontiguousarray(np.eye(BLK, dtype=np.float32))

    common = dict(
        wproj=Wproj_p, w1b=W1b, w2=bf(W2), b1v=b1v,
        bproj=f32(inputs["bproj"]), b2=f32(inputs["b2"]),
        g1=f32(inputs["g1"]), be1=f32(inputs["be1"]),
        g2=f32(inputs["g2"]), be2=f32(inputs["be2"]),
        utri=utri_m, ident=ident_m,
    )
    in_maps = []
    for c in range(NCORE):
        hcols = np.r_[c * HD : (c + 1) * HD, (c + 8) * HD : (c + 9) * HD]
        x_own = np.ascontiguousarray(
            np.concatenate([x[b, j * BLK : (j + 1) * BLK, :] for b, j in _own_blocks(c)])
        )
        in_maps.append(
            dict(
                common,
                x_own=x_own,
                wq=bf(Wq[:, hcols]),
                wk=bf(Wk[:, hcols]),
                wv=bf(Wv[:, hcols]),
            )
        )
    return in_maps


def kernel(**inputs):
    global LAST_RESULT
    in_maps = _prep_inputs(inputs)
    f32v = lambda k: np.asarray(inputs[k], np.float32)
    nc = _build(
        ln1_affine=not (np.all(f32v("g1") == 1) and np.all(f32v("be1") == 0)),
        ln2_affine=not (np.all(f32v("g2") == 1) and np.all(f32v("be2") == 0)),
        add_b2=not np.all(f32v("b2") == 0),
        add_bproj=not np.all(f32v("bproj") == 0),
    )
    res = run_bass_kernel_spmd(
        nc, in_maps, core_ids=list(range(NCORE)), trace=TRACE
    )
    LAST_RESULT = res
    out = np.empty((B, T, C), dtype=np.float32)
    for c in range(NCORE):
        shard = res.results[c]["out"]
        for i, (b, j) in enumerate(_own_blocks(c)):
            out[b, j * BLK : (j + 1) * BLK, :] = shard[i * BLK : (i + 1) * BLK, :]
    return out



# revision 2
# speedup vs baseline: 2.4672x; 2.4672x over previous
"""Trainium2 Bass kernel v2 for the dense transformer block (B=2,T=2048,C=1024,H=16).

Sharding (8 cores): core c handles batch b=c//4 and head-group g=c%4
(heads 4g..4g+3). Each core:
  - streams the FULL batch x (8MB), does LN1 redundantly (4x), transposes
    h1 -> h1T locally (PE, bf16),
  - computes q,k,v for its 4 heads over all 2048 tokens (local, no collective),
  - causal attention for its 4 heads (scores transposed ST[s,t], 4 heads
    packed in one [128,512] psum -> one exp per (jq,j)),
  - partial projection o_part @ Wproj[rows for its 256 o-dims] -> [2048,1024]
    bf16 partial,
  - ONE ReduceScatter(add) within its 4-core batch group -> core at group
    position p owns tokens p*512..(p+1)*512 summed over head groups,
  - residual + LN2 + full MLP on its own 512 tokens (weights streamed).

Single collective (RS ~1MB out/rank) vs baseline's 2 AllGather + AllToAll.
All matmuls bf16 (1 cyc/row), fp32 accumulation; residuals fp32.
"""

import sys

if "/opt/trn_rl_repo" not in sys.path:
    sys.path.insert(0, "/opt/trn_rl_repo")

import ml_dtypes
import numpy as np

import concourse.bass as bass
import concourse.mybir as mybir
import concourse.tile as tile
from concourse import bacc
from concourse.bass_utils import run_bass_kernel_spmd

FP = mybir.dt.float32
BF = mybir.dt.bfloat16
NPBF = ml_dtypes.bfloat16
AF = mybir.ActivationFunctionType
ALU = mybir.AluOpType

B, T, C, H, HD = 2, 2048, 1024, 16, 64
NCORE = 8
GRP = 4              # cores per batch group == head groups == token groups
NHC = 4              # heads per core
BLK = 128
NB = T // BLK        # 16 token blocks per batch
OWN = T // GRP       # 512 tokens owned post-RS
EPS = 1e-5

TRACE = False
LAST_RESULT = None
SIM_MODE = False     # replace the collective with a local DMA (TimelineSim)
NO_PV = False        # debug: skip PV/boundary (st+exp only)
ST_SPLIT = True      # per-head st tiles (sub-region group writes hang HW)


def _bcast(handle, n_free):
    ap = handle[:]
    return bass.AP(tensor=ap.tensor, offset=ap.offset, ap=[[0, 128], *ap.ap])


def _layernorm(nc, pool_stats, eps_sb, out_ap, in_ap, g_sb, be_sb):
    """LN over free axis (1024) of a [128, 1024] tile; out may differ dtype."""
    x3 = in_ap.rearrange("p (n s) -> p n s", s=512)
    stats = pool_stats.tile([128, 2, 6], FP, tag="bnstats")
    for sg in range(2):
        nc.vector.bn_stats(out=stats[:, sg, :], in_=x3[:, sg, :])
    mv = pool_stats.tile([128, 2], FP, tag="bnaggr")
    nc.vector.bn_aggr(out=mv, in_=stats)
    std = pool_stats.tile([128, 1], FP, tag="std")
    nc.scalar.activation(out=std, in_=mv[:, 1:2], func=AF.Sqrt, bias=eps_sb)
    rstd = pool_stats.tile([128, 1], FP, tag="rstd")
    nc.vector.reciprocal(out=rstd, in_=std)
    nc.vector.tensor_scalar(
        out=out_ap,
        in0=in_ap,
        scalar1=mv[:, 0:1],
        scalar2=rstd,
        op0=ALU.subtract,
        op1=ALU.mult,
    )
    if g_sb is not None:
        nc.vector.tensor_mul(out=out_ap, in0=out_ap, in1=g_sb)
    if be_sb is not None:
        nc.vector.tensor_add(out=out_ap, in0=out_ap, in1=be_sb)


def _build(reps=1, ln1_affine=True, ln2_affine=True, add_b2=True, add_bproj=True, stage=4):
    nc = bacc.Bacc(None, num_devices=NCORE)

    xb = nc.dram_tensor("xb", [T, C], FP, kind="ExternalInput")
    x_own = nc.dram_tensor("x_own", [OWN, C], FP, kind="ExternalInput")
    wq = nc.dram_tensor("wq", [C, NHC * HD], BF, kind="ExternalInput")
    wk = nc.dram_tensor("wk", [C, NHC * HD], BF, kind="ExternalInput")
    wv = nc.dram_tensor("wv", [C, NHC * HD], BF, kind="ExternalInput")
    wproj = nc.dram_tensor("wproj", [NHC * HD, C], BF, kind="ExternalInput")
    w1b = nc.dram_tensor("w1b", [32, C, 128], BF, kind="ExternalInput")
    w2 = nc.dram_tensor("w2", [4 * C, C], BF, kind="ExternalInput")
    b1t = nc.dram_tensor("b1t", [128, 32], FP, kind="ExternalInput")
    bproj = nc.dram_tensor("bproj", [C], FP, kind="ExternalInput")
    b2 = nc.dram_tensor("b2", [C], FP, kind="ExternalInput")
    g1 = nc.dram_tensor("g1", [C], FP, kind="ExternalInput")
    be1 = nc.dram_tensor("be1", [C], FP, kind="ExternalInput")
    g2 = nc.dram_tensor("g2", [C], FP, kind="ExternalInput")
    be2 = nc.dram_tensor("be2", [C], FP, kind="ExternalInput")
    utri = nc.dram_tensor("utri", [BLK, BLK], BF, kind="ExternalInput")
    identb = nc.dram_tensor("identb", [BLK, BLK], BF, kind="ExternalInput")
    out = nc.dram_tensor("out", [OWN, C], FP, kind="ExternalOutput")

    rg = [[0, 1, 2, 3], [4, 5, 6, 7]]

    with tile.TileContext(nc) as tc:
        with (
            tc.tile_pool(name="dram", bufs=1, space="DRAM") as dram,
            tc.tile_pool(name="consts", bufs=1) as consts,
            tc.tile_pool(name="stats", bufs=12) as stats,
            tc.tile_pool(name="x2p", bufs=4) as x2p,
            tc.tile_pool(name="tp_ps", bufs=2, space="PSUM") as tp_ps,
        ):
            # ---- constants ----
            eps_sb = consts.tile([128, 1], FP)
            nc.vector.memset(eps_sb, EPS)
            g1b = be1b = g2b = be2b = bprojb = b2b = None
            if ln1_affine:
                g1b = consts.tile([128, C], FP)
                nc.gpsimd.dma_start(out=g1b, in_=_bcast(g1, C))
                be1b = consts.tile([128, C], FP)
                nc.gpsimd.dma_start(out=be1b, in_=_bcast(be1, C))
            if ln2_affine:
                g2b = consts.tile([128, C], FP)
                nc.gpsimd.dma_start(out=g2b, in_=_bcast(g2, C))
                be2b = consts.tile([128, C], FP)
                nc.gpsimd.dma_start(out=be2b, in_=_bcast(be2, C))
            if add_bproj:
                bprojb = consts.tile([128, C], FP)
                nc.gpsimd.dma_start(out=bprojb, in_=_bcast(bproj, C))
            if add_b2:
                b2b = consts.tile([128, C], FP)
                nc.gpsimd.dma_start(out=b2b, in_=_bcast(b2, C))
            utri_sb = consts.tile([BLK, BLK], BF)
            nc.sync.dma_start(out=utri_sb, in_=utri[:])
            utri4_sb = consts.tile([BLK, NHC * BLK], BF)
            for h in range(NHC):
                nc.vector.tensor_copy(
                    out=utri4_sb[:, h * BLK : (h + 1) * BLK], in_=utri_sb
                )
            ident_sb = consts.tile([BLK, BLK], BF)
            nc.sync.dma_start(out=ident_sb, in_=identb[:])
            b1_sb = consts.tile([128, 32], FP)
            nc.gpsimd.dma_start(out=b1_sb, in_=b1t[:])

            def _body(rep):
                rs_in = dram.tile([T, C], BF, name=f"rs_in{rep}", tag=f"ri{rep}")
                rs_out = dram.tile([OWN, C], BF, name=f"rs_out{rep}", tag=f"ro{rep}")

                # residual x for own tokens (also receives proj + ff)
                x2_sb = []
                for i in range(4):
                    x2 = x2p.tile([128, C], FP, tag="x2", name=f"x2_{i}")
                    nc.sync.dma_start(
                        out=x2, in_=x_own[i * 128 : (i + 1) * 128, :]
                    )
                    x2_sb.append(x2)

                with tc.tile_pool(name="uT", bufs=32) as uTp:
                  with tc.tile_pool(name="w1p", bufs=12) as w1p:
                    with (
                        tc.tile_pool(name="wqkvP", bufs=1) as wqkvP,
                        tc.tile_pool(name="qkT", bufs=4) as qkTp,
                        tc.tile_pool(name="vvP", bufs=1) as vvP,
                        tc.tile_pool(name="otb", bufs=4) as otbp,
                        tc.tile_pool(name="oblkP", bufs=3) as oblkP,
                        tc.tile_pool(name="rsb", bufs=2) as rsbp,
                    ):
                        wq_sb = wqkvP.tile([128, 8, NHC * HD], BF, tag="wq")
                        nc.sync.dma_start(
                            out=wq_sb, in_=wq[:].rearrange("(a p) m -> p a m", p=128)
                        )
                        wk_sb = wqkvP.tile([128, 8, NHC * HD], BF, tag="wk")
                        nc.sync.dma_start(
                            out=wk_sb, in_=wk[:].rearrange("(a p) m -> p a m", p=128)
                        )
                        wv_sb = wqkvP.tile([128, 8, NHC * HD], BF, tag="wv")
                        nc.sync.dma_start(
                            out=wv_sb, in_=wv[:].rearrange("(a p) m -> p a m", p=128)
                        )
                        wp_sb = wqkvP.tile([128, 2, C], BF, tag="wp")
                        nc.sync.dma_start(
                            out=wp_sb, in_=wproj[:].rearrange("(a p) m -> p a m", p=128)
                        )
                        # W1 prefetch: only ring-depth (12) chunks may be
                        # issued on the gpsimd queue BEFORE the RS collective
                        # (blocked ring slots ahead of RS deadlock the queue);
                        # the rest are emitted after the RS.
                        w1_sb = []

                        def _w1_load(ut):
                            w1t = w1p.tile(
                                [128, 8, 128], BF, tag="w1", name=f"w1_{ut}"
                            )
                            nc.gpsimd.dma_start(
                                out=w1t,
                                in_=w1b[ut, :, :].rearrange(
                                    "(a p) m -> p a m", p=128
                                ),
                            )
                            return w1t

                        # qT/kT: [128 (2 heads x 64d), T] per head-pair hp
                        qT_sb = [
                            qkTp.tile([128, T], BF, tag="qT", name=f"qT{hp}")
                            for hp in range(2)
                        ]
                        kT_sb = [
                            qkTp.tile([128, T], BF, tag="kT", name=f"kT{hp}")
                            for hp in range(2)
                        ]
                        # v: [128 s, block j, head h, HD+1] (ones col for denom)
                        vv = vvP.tile([128, NB, NHC, HD + 1], BF, tag="vv")
                        nc.vector.memset(vv[:, :, :, HD : HD + 1], 1.0)

                        with (
                            tc.tile_pool(name="xw", bufs=3) as xw,
                            tc.tile_pool(name="h1w", bufs=3) as h1w,
                            tc.tile_pool(name="h1T", bufs=24) as h1Tp,
                            tc.tile_pool(name="ps_a", bufs=4, space="PSUM") as ps_a,
                            tc.tile_pool(name="o_ps", bufs=2, space="PSUM") as o_psP,
                            tc.tile_pool(name="pt", bufs=18) as ptp,
                        ):
                            h1T_t = {}  # block a -> list of 8 ct tiles

                            def emit_Ablock(a):
                                gi, bi = a // 4, a % 4
                                if bi == 0:
                                    h1T_t[gi] = [
                                        h1Tp.tile(
                                            [128, 512], BF, tag="h1T",
                                            name=f"h1T{gi}_{ct}",
                                        )
                                        for ct in range(8)
                                    ]
                                xblk = xw.tile([128, C], FP, tag="x", name=f"x{a}")
                                nc.sync.dma_start(
                                    out=xblk, in_=xb[a * 128 : (a + 1) * 128, :]
                                )
                                h1 = h1w.tile([128, C], BF, tag="h1", name=f"h1_{a}")
                                _layernorm(
                                    nc, stats, eps_sb, h1[:], xblk[:], g1b, be1b
                                )
                                for ct in range(8):
                                    tp = tp_ps.tile([128, 128], BF, tag="tp")
                                    nc.tensor.transpose(
                                        tp, h1[:, ct * 128 : (ct + 1) * 128], ident_sb
                                    )
                                    dst = h1T_t[gi][ct][:, bi * 128 : (bi + 1) * 128]
                                    if ct % 2 == 0:
                                        nc.vector.tensor_copy(out=dst, in_=tp)
                                    else:
                                        nc.scalar.copy(out=dst, in_=tp)

                            def emit_QKV(gi):
                                for wsb, dst in ((wq_sb, qT_sb), (wk_sb, kT_sb)):
                                    for hp in range(2):
                                        ps = ps_a.tile([128, 512], FP, tag="mm", name="ps")
                                        for ct in range(8):
                                            nc.tensor.matmul(
                                                ps,
                                                wsb[:, ct, hp * 128 : (hp + 1) * 128],
                                                h1T_t[gi][ct],
                                                start=(ct == 0),
                                                stop=(ct == 7),
                                            )
                                        dsl = dst[hp][:, gi * 512 : (gi + 1) * 512]
                                        if hp == 0:
                                            nc.vector.tensor_copy(out=dsl, in_=ps)
                                        else:
                                            nc.scalar.copy(out=dsl, in_=ps)
                                for bi in range(4):
                                    a = gi * 4 + bi
                                    vps_t = ps_a.tile([128, 512], FP, tag="mm", name="vps_t")
                                    vps = vps_t[:, 0 : NHC * HD]
                                    for ct in range(8):
                                        nc.tensor.matmul(
                                            vps,
                                            h1T_t[gi][ct][:, bi * 128 : (bi + 1) * 128],
                                            wv_sb[:, ct, :],
                                            start=(ct == 0),
                                            stop=(ct == 7),
                                        )
                                    for hp in range(2):
                                        src = vps[
                                            :, hp * 128 : (hp + 1) * 128
                                        ].rearrange("p (h d) -> p h d", h=2)
                                        dst = vv[:, a, 2 * hp : 2 * hp + 2, 0:HD]
                                        if hp == 0:
                                            nc.vector.tensor_copy(out=dst, in_=src)
                                        else:
                                            nc.scalar.copy(out=dst, in_=src)

                            o_ps_map = {}

                            def emit_st_exp(jq, j):
                                qsl = slice(jq * 128, (jq + 1) * 128)
                                jsl = slice(j * 128, (j + 1) * 128)
                                pt = ptp.tile([128, NHC * BLK], BF, tag="pt")
                                if ST_SPLIT:
                                    for h in range(NHC):
                                        hp, hr = h // 2, (h % 2) * HD
                                        sth = ps_a.tile(
                                            [128, 512], FP, tag="mm", name="sth"
                                        )
                                        nc.tensor.matmul(
                                            sth[:, 0:128],
                                            kT_sb[hp][hr : hr + HD, jsl],
                                            qT_sb[hp][hr : hr + HD, qsl],
                                            start=True,
                                            stop=True,
                                        )
                                        nc.scalar.activation(
                                            out=pt[:, h * 128 : (h + 1) * 128],
                                            in_=sth[:, 0:128],
                                            func=AF.Exp,
                                            scale=0.125,
                                        )
                                else:
                                    st = ps_a.tile(
                                        [128, NHC * BLK], FP, tag="mm", name="st"
                                    )
                                    for h in range(NHC):
                                        hp, hr = h // 2, (h % 2) * HD
                                        nc.tensor.matmul(
                                            st[:, h * 128 : (h + 1) * 128],
                                            kT_sb[hp][hr : hr + HD, jsl],
                                            qT_sb[hp][hr : hr + HD, qsl],
                                            start=(h == 0),
                                            stop=(h == 3),
                                        )
                                    nc.scalar.activation(
                                        out=pt, in_=st, func=AF.Exp, scale=0.125
                                    )
                                if j == jq:
                                    nc.vector.tensor_mul(
                                        out=pt, in0=pt, in1=utri4_sb
                                    )
                                return pt

                            def emit_pv(jq, j, pt):
                                o_ps = o_ps_map[jq]
                                for h in range(NHC):
                                    nc.tensor.matmul(
                                        o_ps[:, h, 0 : HD + 1],
                                        pt[:, h * 128 : (h + 1) * 128],
                                        vv[:, j, h, :],
                                        start=(j == 0 and h == 0),
                                        stop=(j == jq and h == 3),
                                    )

                            def emit_boundary(jq):
                                """softmax scale + oT + partial proj -> rs_in."""
                                o_ps = o_ps_map.pop(jq)
                                recip4 = stats.tile([128, NHC], FP, tag="recip")
                                nc.vector.reciprocal(
                                    out=recip4, in_=o_ps[:, :, HD : HD + 1]
                                )
                                oblk = oblkP.tile([128, NHC * HD], BF, tag="oblk")
                                for h in range(NHC):
                                    nc.vector.tensor_scalar_mul(
                                        out=oblk[:, h * HD : (h + 1) * HD],
                                        in0=o_ps[:, h, 0:HD],
                                        scalar1=recip4[:, h : h + 1],
                                    )
                                ots = []
                                for hp in range(2):
                                    tp = tp_ps.tile([128, 128], BF, tag="tp")
                                    nc.tensor.transpose(
                                        tp,
                                        oblk[:, hp * 128 : (hp + 1) * 128],
                                        ident_sb,
                                    )
                                    ot = otbp.tile(
                                        [128, 128], BF, tag="ot", name=f"ot{jq}_{hp}"
                                    )
                                    if hp == 0:
                                        nc.vector.tensor_copy(out=ot, in_=tp)
                                    else:
                                        nc.scalar.copy(out=ot, in_=tp)
                                    ots.append(ot)
                                rsb = rsbp.tile([128, C], BF, tag="rsb")
                                for co in range(2):
                                    pp = ps_a.tile([128, 512], FP, tag="mm", name="pp")
                                    for hp in range(2):
                                        nc.tensor.matmul(
                                            pp,
                                            ots[hp],
                                            wp_sb[:, hp, co * 512 : (co + 1) * 512],
                                            start=(hp == 0),
                                            stop=(hp == 1),
                                        )
                                    dsl = rsb[:, co * 512 : (co + 1) * 512]
                                    if co == 0:
                                        nc.vector.tensor_copy(out=dsl, in_=pp)
                                    else:
                                        nc.scalar.copy(out=dsl, in_=pp)
                                nc.sync.dma_start(
                                    out=rs_in[jq * 128 : (jq + 1) * 128, :], in_=rsb
                                )

                            # QKV emission units for spreading across jq slots:
                            # 4 q/k chains + 4 v-block chains per group -> 12
                            def qkv_units(gi):
                                units = []
                                for wsb, dst in ((wq_sb, qT_sb), (wk_sb, kT_sb)):
                                    for hp in range(2):
                                        units.append(
                                            lambda gi=gi, wsb=wsb, dst=dst, hp=hp:
                                            emit_qk_chain(gi, wsb, dst, hp)
                                        )
                                for bi in range(4):
                                    units.append(
                                        lambda gi=gi, bi=bi: emit_v_chain(gi, bi)
                                    )
                                return units

                            def emit_qk_chain(gi, wsb, dst, hp):
                                ps = ps_a.tile([128, 512], FP, tag="mm", name="ps")
                                for ct in range(8):
                                    nc.tensor.matmul(
                                        ps,
                                        wsb[:, ct, hp * 128 : (hp + 1) * 128],
                                        h1T_t[gi][ct],
                                        start=(ct == 0),
                                        stop=(ct == 7),
                                    )
                                dsl = dst[hp][:, gi * 512 : (gi + 1) * 512]
                                if hp == 0:
                                    nc.vector.tensor_copy(out=dsl, in_=ps)
                                else:
                                    nc.scalar.copy(out=dsl, in_=ps)

                            def emit_v_chain(gi, bi):
                                a = gi * 4 + bi
                                vps_t = ps_a.tile([128, 512], FP, tag="mm", name="vps_t")
                                vps = vps_t[:, 0 : NHC * HD]
                                for ct in range(8):
                                    nc.tensor.matmul(
                                        vps,
                                        h1T_t[gi][ct][:, bi * 128 : (bi + 1) * 128],
                                        wv_sb[:, ct, :],
                                        start=(ct == 0),
                                        stop=(ct == 7),
                                    )
                                for hp in range(2):
                                    vsrc = vps[
                                        :, hp * 128 : (hp + 1) * 128
                                    ].rearrange("p (h d) -> p h d", h=2)
                                    dst = vv[:, a, 2 * hp : 2 * hp + 2, 0:HD]
                                    if hp == 0:
                                        nc.vector.tensor_copy(out=dst, in_=vsrc)
                                    else:
                                        nc.scalar.copy(out=dst, in_=vsrc)

                            # ---- pipelined emission ----
                            # bootstrap: blocks of groups 0 and 1, QKV(0)
                            for a in range(8):
                                emit_Ablock(a)
                            for u in qkv_units(0):
                                u()

                            # injections[jq]: deferred work emitted right after
                            # PV of jq completes (spread across the pipeline)
                            injections = {jq: [] for jq in range(NB)}
                            nw1 = 0
                            for gi in range(4):
                                for ii in range(4):
                                    jq = gi * 4 + ii
                                    # A-blocks of group gi+2 during attn(gi)
                                    na = (gi + 2) * 4 + ii
                                    if na < NB:
                                        injections[jq].append(
                                            lambda na=na: emit_Ablock(na)
                                        )
                                    # QKV chains of group gi+1 spread over attn(gi)
                                    if gi < 3:
                                        units = qkv_units(gi + 1)
                                        per = [units[0:3], units[3:6], units[6:9], units[9:12]][ii]
                                        injections[jq].extend(per)
                                    # W1 prefetch: 2 chunks per jq, only
                                    # up to the ring depth before the RS
                                    for _ in range(2):
                                        if nw1 < 12:
                                            ut = nw1
                                            injections[jq].append(
                                                lambda ut=ut: w1_sb.append(
                                                    _w1_load(ut)
                                                )
                                            )
                                            nw1 += 1

                            if stage < 2:
                                for a in range(8, 16):
                                    emit_Ablock(a)
                                for gg in range(1, 4):
                                    for u in qkv_units(gg):
                                        u()
                                for ut in range(32):
                                    w1_sb.append(_w1_load(ut))
                            else:
                                steps = [
                                    (jq, j)
                                    for jq in range(NB)
                                    for j in range(jq + 1)
                                ]
                                pts_map = {jq: [] for jq in range(NB)}

                                def flush(q):
                                    o_ps_map[q] = o_psP.tile(
                                        [128, NHC, 72], FP, tag="o", name=f"o{q}"
                                    )
                                    for j2, ppt in enumerate(pts_map[q]):
                                        emit_pv(q, j2, ppt)
                                    emit_boundary(q)
                                    for fn in injections[q]:
                                        fn()
                                    pts_map[q] = None

                                pending_jq = None
                                for jq, j in steps:
                                    pt = emit_st_exp(jq, j)
                                    pts_map[jq].append(pt)
                                    if NO_PV:
                                        pts_map[jq] = []
                                        continue
                                    if pending_jq is not None:
                                        flush(pending_jq)
                                        pending_jq = None
                                    if j == jq:
                                        pending_jq = jq
                                if not NO_PV:
                                    flush(pending_jq)
                                else:
                                    for q in range(NB):
                                        for fn in injections[q]:
                                            fn()

                    # ---- ReduceScatter within the 4-core batch group ----
                    if stage < 3:
                        pass
                    elif SIM_MODE:
                        nc.sync.dma_start(out=rs_out[:], in_=rs_in[0:OWN, :])
                    else:
                        nc.gpsimd.collective_compute(
                            "ReduceScatter",
                            ALU.add,
                            replica_groups=rg,
                            ins=[rs_in[:].opt()],
                            outs=[rs_out[:].opt()],
                        )

                    # remaining W1 chunks (ring now drains as up consumes)
                    for ut in range(12, 32):
                        w1_sb.append(_w1_load(ut))

                    # ---- post-RS: residual + LN2 + h2T ----
                    if stage < 3:
                        for i in range(4):
                            nc.sync.dma_start(
                                out=out[i * 128 : (i + 1) * 128, :], in_=x2_sb[i]
                            )
                    with (
                        tc.tile_pool(name="rso", bufs=2) as rsop,
                        tc.tile_pool(name="h2w", bufs=2) as h2w,
                        tc.tile_pool(name="h2T", bufs=8) as h2Tp,
                    ):
                        h2T_sb = [
                            h2Tp.tile([128, OWN], BF, tag="h2T", name=f"h2T{ct}")
                            for ct in range(8)
                        ] if stage >= 3 else []
                        for i in range(4) if stage >= 3 else []:
                            rso = rsop.tile([128, C], BF, tag="rso")
                            nc.sync.dma_start(
                                out=rso, in_=rs_out[i * 128 : (i + 1) * 128, :]
                            )
                            nc.vector.tensor_add(
                                out=x2_sb[i], in0=x2_sb[i], in1=rso
                            )
                            if add_bproj:
                                nc.vector.tensor_add(
                                    out=x2_sb[i], in0=x2_sb[i], in1=bprojb
                                )
                            h2 = h2w.tile([128, C], BF, tag="h2", name=f"h2_{i}")
                            _layernorm(
                                nc, stats, eps_sb, h2[:], x2_sb[i][:], g2b, be2b
                            )
                            for ct in range(8):
                                tp = tp_ps.tile([128, 128], BF, tag="tp")
                                nc.tensor.transpose(
                                    tp, h2[:, ct * 128 : (ct + 1) * 128], ident_sb
                                )
                                dst = h2T_sb[ct][:, i * 128 : (i + 1) * 128]
                                if ct % 2 == 0:
                                    nc.vector.tensor_copy(out=dst, in_=tp)
                                else:
                                    nc.scalar.copy(out=dst, in_=tp)

                        # ---- MLP up (W1 prefetched) ----
                        uT_sb = []
                        with tc.tile_pool(
                            name="up_ps", bufs=2, space="PSUM"
                        ) as up_ps:
                            for ut in range(32) if stage >= 3 else []:
                                ups = up_ps.tile([128, OWN], FP, tag="up")
                                for ct in range(8):
                                    nc.tensor.matmul(
                                        ups,
                                        w1_sb[ut][:, ct, :],
                                        h2T_sb[ct],
                                        start=(ct == 0),
                                        stop=(ct == 7),
                                    )
                                u = uTp.tile(
                                    [128, OWN], BF, tag="uT", name=f"uT{ut}"
                                )
                                nc.scalar.activation(
                                    out=u,
                                    in_=ups,
                                    func=AF.Relu,
                                    bias=b1_sb[:, ut : ut + 1],
                                )
                                uT_sb.append(u)

                  # ---- MLP down (W2 re-streamed per half, ring 16) ----
                  if stage == 3:
                      for i in range(4):
                          nc.sync.dma_start(
                              out=out[i * 128 : (i + 1) * 128, :], in_=x2_sb[i]
                          )
                  with (
                      tc.tile_pool(name="w2p", bufs=16) as w2p,
                      tc.tile_pool(name="dn_ps", bufs=4, space="PSUM") as dn_ps,
                  ):
                      if stage >= 4:
                          w2_sb = []
                          for kk in range(32):
                              w2t = w2p.tile(
                                  [128, C], BF, tag="w2", name=f"w2_{kk}"
                              )
                              nc.gpsimd.dma_start(
                                  out=w2t, in_=w2[kk * 128 : (kk + 1) * 128, :]
                              )
                              w2_sb.append(w2t)
                          for g in range(4):
                              for tq in range(4):
                                  for co in range(2):
                                      dn = dn_ps.tile(
                                          [128, 512], FP, tag="dn", name="dn"
                                      )
                                      for k in range(8):
                                          kk = g * 8 + k
                                          nc.tensor.matmul(
                                              dn,
                                              uT_sb[kk][:, tq * 128 : (tq + 1) * 128],
                                              w2_sb[kk][:, co * 512 : (co + 1) * 512],
                                              start=(k == 0),
                                              stop=(k == 7),
                                          )
                                      csl = slice(co * 512, (co + 1) * 512)
                                      nc.vector.tensor_add(
                                          out=x2_sb[tq][:, csl],
                                          in0=x2_sb[tq][:, csl],
                                          in1=dn,
                                      )
                              if g == 3:
                                  for tq in range(4):
                                      if add_b2:
                                          nc.vector.tensor_add(
                                              out=x2_sb[tq], in0=x2_sb[tq], in1=b2b
                                          )
                                      nc.sync.dma_start(
                                          out=out[tq * 128 : (tq + 1) * 128, :],
                                          in_=x2_sb[tq],
                                      )

            for _rep in range(reps):
                _body(_rep)

    nc.compile()
    return nc


def _prep_inputs(inputs):
    f32 = lambda a: np.ascontiguousarray(np.asarray(a, dtype=np.float32))
    bf = lambda a: np.ascontiguousarray(np.asarray(a, np.float32).astype(NPBF))
    x = f32(inputs["x"])
    Wq2 = np.asarray(inputs["Wq"], np.float32).transpose(1, 0, 2).reshape(C, C)
    Wk2 = np.asarray(inputs["Wk"], np.float32).transpose(1, 0, 2).reshape(C, C)
    Wv2 = np.asarray(inputs["Wv"], np.float32).transpose(1, 0, 2).reshape(C, C)
    Wproj = np.asarray(inputs["Wproj"], np.float32)
    W1 = np.asarray(inputs["W1"], np.float32)
    W2 = np.asarray(inputs["W2"], np.float32)

    common = dict(
        w1b=bf(W1.reshape(C, 32, 128).transpose(1, 0, 2)),
        w2=bf(W2),
        b1t=f32(np.asarray(inputs["b1"], np.float32).reshape(32, 128).T),
        bproj=f32(inputs["bproj"]),
        b2=f32(inputs["b2"]),
        g1=f32(inputs["g1"]),
        be1=f32(inputs["be1"]),
        g2=f32(inputs["g2"]),
        be2=f32(inputs["be2"]),
        utri=np.ascontiguousarray(
            np.triu(np.ones((BLK, BLK), np.float32)).astype(NPBF)
        ),
        identb=np.ascontiguousarray(np.eye(BLK, dtype=np.float32).astype(NPBF)),
    )
    in_maps = []
    for c in range(NCORE):
        b, g = c // GRP, c % GRP
        p = c % GRP
        cs = slice(g * NHC * HD, (g + 1) * NHC * HD)
        in_maps.append(
            dict(
                common,
                xb=f32(x[b]),
                x_own=f32(x[b, p * OWN : (p + 1) * OWN]),
                wq=bf(Wq2[:, cs]),
                wk=bf(Wk2[:, cs]),
                wv=bf(Wv2[:, cs]),
                wproj=bf(Wproj[cs, :]),
            )
        )
    return in_maps


def kernel(**inputs):
    global LAST_RESULT
    in_maps = _prep_inputs(inputs)
    f32v = lambda k: np.asarray(inputs[k], np.float32)
    nc = _build(
        ln1_affine=not (np.all(f32v("g1") == 1) and np.all(f32v("be1") == 0)),
        ln2_affine=not (np.all(f32v("g2") == 1) and np.all(f32v("be2") == 0)),
        add_b2=not np.all(f32v("b2") == 0),
        add_bproj=not np.all(f32v("bproj") == 0),
    )
    res = run_bass_kernel_spmd(
        nc, in_maps, core_ids=list(range(NCORE)), trace=TRACE
    )
    LAST_RESULT = res
    outa = np.empty((B, T, C), dtype=np.float32)
    for c in range(NCORE):
        b, p = c // GRP, c % GRP
        outa[b, p * OWN : (p + 1) * OWN, :] = res.results[c]["out"]
    return outa
